# revision 1
# baseline (speedup 1.0000x reference)
"""Trainium2 Bass kernel for ViTDet-style attention with decomposed
relative-position bias.

Problem shapes (hardcoded):
  x: (4, 32, 32, 768) f32, Wqkv: (768, 2304), Wproj: (768, 768),
  bproj: (768,), rel_pos_h/w: (63, 64).
  12 heads, head_dim 64, S = 32*32 = 1024.

Sharding: 48 (batch, head) pairs -> 6 heads per core, all of one batch per
core-pair. Each core computes its heads' attention and a partial output
projection (its heads' channel rows of Wproj); the host sums the two
partials per batch and adds bproj.

Device algorithm per core (bf16 matmuls, fp32 PSUM accumulation):
  - qkT = Wqk^T @ x^T  (x^T supplied pre-transposed by host; k pre-scaled)
  - v   = x @ Wv       (natural layout, with an appended ones column)
  - PhT = rel_pos_h^T @ qT; band-extract BhT[kh',(h,w)] = PhT[kh'+h,(h,w)]
    on the PE via shifted-identity selection matmuls (same for W axis)
  - scoresT (k x q) = kaugT^T @ qaugT in ONE K=128 matmul per tile:
    aug rows 0-63 = kT / qT, 64-95 = one-hot(h) / BhT, 96-127 = one-hot(w)/BwT
    => rel-pos bias folded into the QK matmul for free.
  - eT = exp(scoresT) on ScalarE (no max subtraction; scores are O(1)).
  - avT (65 x q) accumulates v_aug^T-matmul over k blocks; row 64 = softmax
    denominator via the ones column.
  - normalize via DVE reciprocal + gpsimd partition-broadcast + DVE multiply.
  - partial = out_heads @ Wproj_shard  (natural layout, DMA PSUM->DRAM).
"""

import numpy as np

import concourse.bass as bass
import concourse.bacc as bacc
import concourse.mybir as mybir
import concourse.tile as tile
from concourse.bass_utils import run_bass_kernel_spmd

F32 = mybir.dt.float32
F32R = mybir.dt.float32r
BF16 = mybir.dt.bfloat16

NH = 12          # total heads
C = 768
HD = 64
H = W = 32
S = H * W        # 1024
B = 4
NCORES = 8
HPC = NH * B // NCORES   # heads per core = 6
NCH = 6                  # C // 128 input-channel chunks
NKB = S // 128           # 8 k blocks
NQB = S // 128           # 8 q blocks
NHALF = 512              # matmul moving-dim half


def _r(ap):
    # operands are already float32r-typed
    return ap


def build_program():
    nc = bacc.Bacc("TRN2", target_bir_lowering=False, debug=False)

    xT = nc.declare_dram_parameter("xT", [C, S], BF16, isOutput=False)
    wqk = nc.declare_dram_parameter("wqk", [C, 2 * HPC * HD], BF16, isOutput=False)
    wv = nc.declare_dram_parameter("wv", [C, HPC * HD], BF16, isOutput=False)
    wproj = nc.declare_dram_parameter("wproj", [HPC * HD, C], BF16, isOutput=False)
    rhT = nc.declare_dram_parameter("rhT", [HD, 2 * H - 1], BF16, isOutput=False)
    rwT = nc.declare_dram_parameter("rwT", [HD, 2 * W - 1], BF16, isOutput=False)
    onehot = nc.declare_dram_parameter("onehot", [65, S], BF16, isOutput=False)
    idband = nc.declare_dram_parameter("idband", [2 * H - 1, 3 * W - 1], BF16,
                                       isOutput=False)
    out = nc.declare_dram_parameter("out", [S, C], F32, isOutput=True)

    # small DRAM bounce buffers for the rowsum transpose (I/O tensors --
    # internal DRAM scratch is paged and much slower for strided DMAs)
    rs_dram = nc.declare_dram_parameter("rs_dram", [S], F32, isOutput=True)
    rc_dram = nc.declare_dram_parameter("rc_dram", [S], F32, isOutput=True)

    with tile.TileContext(nc) as tc:
        with (
            tc.tile_pool(name="persist", bufs=1) as persist,
            tc.tile_pool(name="psum_big", bufs=2, space="PSUM") as psum_big,
            tc.tile_pool(name="psum_av", bufs=2, space="PSUM") as psum_av,
            tc.tile_pool(name="et", bufs=3) as et_pool,
            tc.tile_pool(name="small", bufs=2) as small,
        ):
            # ---- persistent SBUF loads ----
            xT_sb = []
            for ci in range(NCH):
                t = persist.tile([128, S], BF16, tag=f"xT{ci}", name=f"xT{ci}")
                nc.sync.dma_start(t[:], xT[128 * ci:128 * (ci + 1), :])
                xT_sb.append(t)
            wqk_sb = []
            for ci in range(NCH):
                t = persist.tile([128, 2 * HPC * HD], BF16, tag=f"wqk{ci}", name=f"wqk{ci}")
                nc.sync.dma_start(t[:], wqk[128 * ci:128 * (ci + 1), :])
                wqk_sb.append(t)
            wv_sb = []
            for ci in range(NCH):
                t = persist.tile([128, HPC * HD], BF16, tag=f"wv{ci}", name=f"wv{ci}")
                nc.sync.dma_start(t[:], wv[128 * ci:128 * (ci + 1), :])
                wv_sb.append(t)
            wproj_sb = []
            for ci in range(HPC * HD // 128):
                t = persist.tile([128, C], BF16, tag=f"wproj{ci}", name=f"wproj{ci}")
                nc.sync.dma_start(t[:], wproj[128 * ci:128 * (ci + 1), :])
                wproj_sb.append(t)
            idb_sb = persist.tile([2 * H - 1, 3 * W - 1], BF16, tag="idb",
                                  name="idb_sb")
            nc.sync.dma_start(idb_sb[:], idband[:, :])
            rhT_sb = persist.tile([HD, 2 * H - 1], BF16, tag="rhT", name="rhT_sb")
            nc.sync.dma_start(rhT_sb[:], rhT[:, :])
            rwT_sb = persist.tile([HD, 2 * W - 1], BF16, tag="rwT", name="rwT_sb")
            nc.sync.dma_start(rwT_sb[:], rwT[:, :])

            # ---- one-hot template (65, S), host-supplied constant ----
            # rows 0-63: one-hot(h)/one-hot(w) reversed; row 64: all ones
            oh = persist.tile([65, S], BF16, tag="onehot", name="onehot")
            nc.sync.dma_start(oh[:], onehot[:, :])

            # ---- augmented k/q tiles (128, S) per head ----
            kaug = [persist.tile([128, S], BF16, tag=f"kaug{i}", name=f"kaug{i}") for i in range(HPC)]
            qaug = [persist.tile([128, S], BF16, tag=f"qaug{i}", name=f"qaug{i}") for i in range(HPC)]
            for i in range(HPC):
                nc.vector.tensor_copy(kaug[i][64:128, :], oh[0:64, :])

            # ---- v projection (natural) + ones column ----
            # v_sb[sb]: (128, 6*65) cols [65i..65i+64) = head i v, col 65i+64 = 1
            v_sb = [persist.tile([128, HPC * (HD + 1)], BF16, tag=f"v{sb}", name=f"v{sb}")
                    for sb in range(NKB)]
            for sb in range(NKB):
                vp = psum_big.tile([128, HPC * HD + HPC], F32, tag="big", name="vp")
                for ci in range(NCH):
                    nc.tensor.matmul(
                        vp[:, 0:HPC * HD],
                        _r(xT_sb[ci][:, 128 * sb:128 * (sb + 1)]),
                        _r(wv_sb[ci][:]),
                        start=(ci == 0), stop=(ci == NCH - 1))
                # ones columns via outer product of the ones row
                nc.tensor.matmul(vp[:, HPC * HD:HPC * HD + HPC],
                                 oh[64:65, 128 * sb:128 * (sb + 1)],
                                 oh[64:65, 0:HPC], start=True, stop=True)
                src = bass.AP(vp.tensor, vp[:].offset,
                              [vp[:].ap[0], [HD, HPC], [1, HD]])
                dst = bass.AP(v_sb[sb].tensor, v_sb[sb][:].offset,
                              [v_sb[sb][:].ap[0], [HD + 1, HPC], [1, HD]])
                nc.vector.tensor_copy(dst, src)
                ones_src = bass.AP(vp.tensor, vp[:].offset + HPC * HD,
                                   [vp[:].ap[0], [1, HPC]])
                ones_dst = bass.AP(v_sb[sb].tensor, v_sb[sb][:].offset + HD,
                                   [v_sb[sb][:].ap[0], [HD + 1, HPC]])
                nc.vector.tensor_copy(ones_dst, ones_src)

            # ---- qk projection (transposed layout) ----
            # qkT octile t covers oc rows [128t, 128t+128): t<3 -> q, t>=3 -> k
            for t in range(2 * HPC * HD // 128):
                qp = psum_big.tile([128, S], F32, tag="big", name="qp")
                for ci in range(NCH):
                    for nh in range(S // NHALF):
                        nc.tensor.matmul(
                            qp[:, NHALF * nh:NHALF * (nh + 1)],
                            _r(wqk_sb[ci][:, 128 * t:128 * (t + 1)]),
                            _r(xT_sb[ci][:, NHALF * nh:NHALF * (nh + 1)]),
                            start=(ci == 0), stop=(ci == NCH - 1))
                for sub in range(2):
                    head = (t % 3) * 2 + sub
                    dst = (qaug if t < 3 else kaug)[head]
                    if t < 3:
                        nc.scalar.copy(dst[0:64, :], qp[64 * sub:64 * sub + 64, :])
                    else:
                        nc.vector.tensor_copy(dst[0:64, :],
                                              qp[64 * sub:64 * sub + 64, :])

            # ---- per head: rel-pos tables -> band-gather into qaug ----
            for i in range(HPC):
                php = psum_big.tile([2 * H - 1, S], F32, tag="big", name="php")
                pwp = psum_big.tile([2 * W - 1, S], F32, tag="big", name="pwp")
                for nh in range(S // NHALF):
                    sl = slice(NHALF * nh, NHALF * (nh + 1))
                    nc.tensor.matmul(php[:, sl], _r(rhT_sb[:]),
                                     _r(qaug[i][0:64, sl]), start=True, stop=True)
                    nc.tensor.matmul(pwp[:, sl], _r(rwT_sb[:]),
                                     _r(qaug[i][0:64, sl]), start=True, stop=True)
                ph_sb = small.tile([2 * H - 1, S], BF16, tag="ph_sb",
                                   name="ph_sb", bufs=2)
                pw_sb = small.tile([2 * W - 1, S], BF16, tag="pw_sb",
                                   name="pw_sb", bufs=2)
                nc.scalar.copy(ph_sb[:], php[:])
                nc.vector.tensor_copy(pw_sb[:], pwp[:])
                # band-extract on PE: BhT_rev[kh', (h,w)] = PhT[kh'+h, (h,w)]
                # = sum_r idband[r, kh'+h] * PhT[r, (h,w)]  (idband = I_63)
                bhp = psum_big.tile([H, S], F32, tag="big", name="bhp")
                bwp = psum_big.tile([W, S], F32, tag="big", name="bwp")
                for h in range(H):
                    nc.tensor.matmul(bhp[:, W * h:W * (h + 1)],
                                     idb_sb[:, h:h + H],
                                     ph_sb[:, W * h:W * (h + 1)],
                                     start=True, stop=True)
                for w in range(W):
                    # w-major output block: bwp[kw', w*32+h] = PwT[kw'+w,(h,w)]
                    rhs_w = bass.AP(pw_sb.tensor, pw_sb[:].offset + w,
                                    [pw_sb[:].ap[0], [W, H]])
                    nc.tensor.matmul(bwp[:, H * w:H * (w + 1)],
                                     idb_sb[:, w:w + W], rhs_w,
                                     start=True, stop=True)
                nc.scalar.copy(qaug[i][64:96, :], bhp[:])
                # permute w-major back to (h, w) order during the copy
                bwp_perm = bass.AP(bwp.tensor, bwp[:].offset,
                                   [bwp[:].ap[0], [1, H], [H, W]])
                nc.vector.tensor_copy(qaug[i][96:128, :], bwp_perm)

            # ---- attention per head ----
            out_headsT = [persist.tile([128, S], BF16, tag=f"ohT{c}",
                                       name=f"ohT{c}")
                          for c in range(HPC * HD // 128)]
            for i in range(HPC):
                av = psum_av.tile([HD + 1, S], F32, tag="av", name="av")
                for kb in range(NKB):
                    sc = psum_big.tile([128, S], F32, tag="big", name="qp")
                    for nh in range(S // NHALF):
                        sl = slice(NHALF * nh, NHALF * (nh + 1))
                        nc.tensor.matmul(
                            sc[:, sl],
                            _r(kaug[i][:, 128 * kb:128 * (kb + 1)]),
                            _r(qaug[i][:, sl]), start=True, stop=True)
                    e = et_pool.tile([128, S], BF16, tag="et", name="et")
                    nc.scalar.activation(e[:], sc[:],
                                         mybir.ActivationFunctionType.Exp)
                    for nh in range(S // NHALF):
                        sl = slice(NHALF * nh, NHALF * (nh + 1))
                        nc.tensor.matmul(
                            av[:, sl],
                            _r(v_sb[kb][:, (HD + 1) * i:(HD + 1) * (i + 1)]),
                            _r(e[:, sl]),
                            start=(kb == 0), stop=(kb == NKB - 1))
                rowsum = small.tile([1, S], F32, tag="rowsum", name="rowsum",
                                    bufs=1)
                nc.scalar.copy(rowsum[:], av[HD:HD + 1, :])
                nc.sync.dma_start(bass.AP(rs_dram, 0, [[1, S]]), rowsum[:])
                rs_t = small.tile([128, NQB], F32, tag="rs_t", name="rs_t")
                nc.sync.dma_start(
                    rs_t[:], bass.AP(rs_dram, 0, [[1, 128], [128, NQB]]))
                rc_t = small.tile([128, NQB], F32, tag="rc_t", name="rc_t")
                nc.vector.reciprocal(rc_t[:], rs_t[:])
                nc.sync.dma_start(
                    bass.AP(rc_dram, 0, [[1, 128], [128, NQB]]), rc_t[:])
                recip = small.tile([1, S], F32, tag="recip", name="recip",
                                   bufs=1)
                nc.sync.dma_start(recip[:], bass.AP(rc_dram, 0, [[1, S]]))
                rb = small.tile([64, S], F32, tag="rbcast", name="rbcast",
                                bufs=1)
                nc.gpsimd.partition_broadcast(rb[:], recip[:])
                chunk, row = i // 2, (i % 2) * 64
                nc.vector.tensor_tensor(
                    out_headsT[chunk][row:row + 64, :], av[0:HD, :], rb[:],
                    op=mybir.AluOpType.mult)

            # ---- output projection (partial) ----
            for qb in range(NQB):
                pp = psum_big.tile([128, C], F32, tag="big", name="pp")
                for ci in range(HPC * HD // 128):
                    nc.tensor.matmul(
                        pp[:, 0:NHALF],
                        _r(out_headsT[ci][:, 128 * qb:128 * (qb + 1)]),
                        _r(wproj_sb[ci][:, 0:NHALF]),
                        start=(ci == 0), stop=(ci == 2))
                    nc.tensor.matmul(
                        pp[:, NHALF:C],
                        _r(out_headsT[ci][:, 128 * qb:128 * (qb + 1)]),
                        _r(wproj_sb[ci][:, NHALF:C]),
                        start=(ci == 0), stop=(ci == 2))
                pp_sb = small.tile([128, C], F32, tag="pp_sb", name="pp_sb", bufs=1)
                (nc.scalar.copy if qb % 2 else nc.vector.tensor_copy)(
                    pp_sb[:], pp[:])
                nc.sync.dma_start(out[128 * qb:128 * (qb + 1), :], pp_sb[:])

    nc.compile()
    return nc


def shard_inputs(x, Wqkv, Wproj, rel_pos_h, rel_pos_w):
    """Build the 8 per-core input maps."""
    import ml_dtypes
    bf16 = ml_dtypes.bfloat16
    scale = HD ** (-0.5)
    x = np.asarray(x, dtype=np.float32)
    Wqkv = np.asarray(Wqkv, dtype=np.float32)
    Wproj = np.asarray(Wproj, dtype=np.float32)
    rhT = np.ascontiguousarray(np.asarray(rel_pos_h, np.float32).T).astype(bf16)
    rwT = np.ascontiguousarray(np.asarray(rel_pos_w, np.float32).T).astype(bf16)
    idb = np.zeros((2 * H - 1, 3 * W - 1), np.float32)
    for r in range(2 * H - 1):
        idb[r, r] = 1.0
    idb = idb.astype(bf16)
    oh = np.zeros((65, S), np.float32)
    for khp in range(H):
        oh[khp, (31 - khp) * W:(31 - khp) * W + W] = 1.0
    for kwp in range(W):
        oh[32 + kwp, 31 - kwp::W] = 1.0
    oh[64, :] = 1.0
    oh = oh.astype(bf16)
    in_maps = []
    for core in range(NCORES):
        b = core // 2
        h0 = (core % 2) * HPC
        xb = x[b].reshape(S, C)
        xT = np.ascontiguousarray(xb.T).astype(bf16)
        wq = Wqkv[:, h0 * HD:(h0 + HPC) * HD]
        wk = Wqkv[:, C + h0 * HD:C + (h0 + HPC) * HD] * scale
        wqk = np.ascontiguousarray(np.concatenate([wq, wk], axis=1)).astype(bf16)
        wv = np.ascontiguousarray(
            Wqkv[:, 2 * C + h0 * HD:2 * C + (h0 + HPC) * HD]).astype(bf16)
        wp = np.ascontiguousarray(Wproj[h0 * HD:(h0 + HPC) * HD, :]).astype(bf16)
        in_maps.append({"xT": xT, "wqk": wqk, "wv": wv, "wproj": wp,
                        "rhT": rhT, "rwT": rwT, "onehot": oh,
                        "idband": idb})
    return in_maps


_NC_CACHE = {}


def kernel(x, Wqkv, Wproj, bproj, rel_pos_h, rel_pos_w):
    if "nc" not in _NC_CACHE:
        _NC_CACHE["nc"] = build_program()
    nc = _NC_CACHE["nc"]
    in_maps = shard_inputs(x, Wqkv, Wproj, rel_pos_h, rel_pos_w)
    res = run_bass_kernel_spmd(nc, in_maps, list(range(NCORES)))
    bproj = np.asarray(bproj, dtype=np.float32)
    out = np.empty((B, H, W, C), dtype=np.float32)
    for b in range(B):
        acc = res.results[2 * b]["out"] + res.results[2 * b + 1]["out"] + bproj
        out[b] = acc.reshape(H, W, C)
    return out



# revision 10
# speedup vs baseline: 1.4473x; 1.4473x over previous
"""Trainium2 Bass kernel for ViTDet-style attention with decomposed
relative-position bias.

Problem shapes (hardcoded):
  x: (4, 32, 32, 768) f32, Wqkv: (768, 2304), Wproj: (768, 768),
  bproj: (768,), rel_pos_h/w: (63, 64).
  12 heads, head_dim 64, S = 32*32 = 1024.

Sharding: 48 (batch, head) pairs -> 6 heads per core, all of one batch per
core-pair. Each core computes its heads' attention and a partial output
projection (its heads' channel rows of Wproj); the host sums the two
partials per batch and adds bproj.

Device algorithm per core:
  - qkT = Wqk^T @ x^T and v = x @ Wv. With FP8_QKV these use fp8e4
    DoubleRow (two 128-row K tiles per pass, 2x); otherwise bf16.
    Weights are pre-scaled x64 (k additionally by 1/sqrt(hd)) so fp8 stays
    in normal range; the exp activation's scale=2^-12 folds it back out.
  - rel-pos bias band rows computed DIRECTLY per shift: for query row h the
    band tile rows are rhT_flip[:, 31-h:63-h]^T @ q64 — no 63-row table
    intermediate. Two heads per matmul (a matmul's PSUM extent must stay
    inside one 2KB bank), four 256-col quarters per axis.
  - scoresT (k x q) = kaug^T @ qaug in one K=128 bf16 matmul per tile:
    rows 0-63 k8/q64, 64-95 one-hot(kh)/bandH, 96-127 one-hot(kw)/bandW.
  - e = exp(2^-12 * scores) on ScalarE; av accumulates over k blocks with
    v blocks padded to 128 stationary cols (64 v + ones + 63 zeros); row 64
    of av is the softmax denominator via the ones column.
  - normalize: av->SBUF f32 copy (frees PSUM fast), DVE
    reciprocal_approx_fast on the denominator row (plain reciprocal is ~9
    cyc/elem and single-lane here), gpsimd partition-broadcast, DVE mult.
  - partial = out_headsT^T @ (Wproj/64) in bf16, PSUM->SBUF->DRAM.
"""

import numpy as np

import concourse.bass as bass
import concourse.bacc as bacc
import concourse.mybir as mybir
import concourse.tile as tile
from concourse.bass_utils import run_bass_kernel_spmd

F32 = mybir.dt.float32
BF16 = mybir.dt.bfloat16
FP8 = mybir.dt.float8e4

NH = 12          # total heads
C = 768
HD = 64
H = W = 32
S = H * W        # 1024
B = 4
NCORES = 8
HPC = NH * B // NCORES   # heads per core = 6
EXP_SCALE = float(2.0 ** -12)

FP8_QKV = False  # fp8 DoubleRow for the qk/v projections (adds ~1.7e-2 err)


def _ap(t, off, dims):
    return bass.AP(t.tensor, t[:].offset + off, [t[:].ap[0]] + dims)


def _app(t, p0, psz, off, dims):
    # AP with partition offset/size override
    base = t[p0:p0 + psz, :]
    return bass.AP(t.tensor, base.offset + off, [base.ap[0]] + dims)


def build_program():
    nc = bacc.Bacc("TRN2", target_bir_lowering=False, debug=False)

    XDT = FP8 if FP8_QKV else BF16
    xTp = nc.declare_dram_parameter("xTp", [128, 6144], XDT, isOutput=False)
    wqkp = nc.declare_dram_parameter("wqkp", [128, 4608], XDT, isOutput=False)
    wvp = nc.declare_dram_parameter("wvp", [128, 2304], XDT, isOutput=False)
    wpr = nc.declare_dram_parameter("wpr", [384, 768], BF16, isOutput=False)
    rh2 = nc.declare_dram_parameter("rh2", [64, 63], BF16, isOutput=False)
    rw2 = nc.declare_dram_parameter("rw2", [64, 63], BF16, isOutput=False)
    ohk = nc.declare_dram_parameter("ohk", [64, 1024], BF16, isOutput=False)
    out = nc.declare_dram_parameter("out", [S, C], F32, isOutput=True)

    with tile.TileContext(nc) as tc:
        with (
            tc.tile_pool(name="persist", bufs=1) as persist,
            tc.tile_pool(name="ps", bufs=2, space="PSUM") as ps,
            tc.tile_pool(name="small", bufs=2) as small,
        ):
            # ---- persistent SBUF loads (order = need order) ----
            xTp_sb = persist.tile([128, 6144], XDT, tag="xTp", name="xTp_sb")
            nc.sync.dma_start(xTp_sb[:], xTp[:, :])
            wqkp_sb = persist.tile([128, 4608], XDT, tag="wqkp", name="wqkp_sb")
            nc.sync.dma_start(wqkp_sb[:], wqkp[:, :])
            rh2_sb = persist.tile([64, 63], BF16, tag="rh2", name="rh2_sb")
            nc.sync.dma_start(rh2_sb[:], rh2[:, :])
            rw2_sb = persist.tile([64, 63], BF16, tag="rw2", name="rw2_sb")
            nc.sync.dma_start(rw2_sb[:], rw2[:, :])

            # qaug/kaug: per head i at cols [1024i, 1024(i+1)):
            #   rows 0-63 q64 / k8, 64-95 bandH / onehot(kh), 96-127 bandW /
            #   onehot(kw)
            qaug = persist.tile([128, HPC * S], BF16, tag="qaug", name="qaug")
            kaug = persist.tile([128, HPC * S], BF16, tag="kaug", name="kaug")
            for i in range(HPC):
                nc.sync.dma_start(kaug[64:128, S * i:S * (i + 1)], ohk[:, :])

            wvp_sb = persist.tile([128, 2304], XDT, tag="wvp", name="wvp_sb")
            nc.sync.dma_start(wvp_sb[:], wvp[:, :])
            wpr_sb = persist.tile([128, 2304], BF16, tag="wpr", name="wpr_sb")
            nc.sync.dma_start(
                wpr_sb[:],
                bass.AP(wpr, 0, [[768, 128], [128 * 768, 3], [1, 768]]))

            # v in bf16 pair layout: vps[p, j, i, h, d] with j = k-block pair,
            # i = which block of the pair, h = head, d = 128 cols
            # (64 v + ones + 63 zeros)
            vps = persist.tile([128, 4 * 2 * HPC * 128], BF16, tag="vps",
                               name="vps")
            ohT = persist.tile([128, 3 * S], BF16, tag="ohT", name="ohT")

            DR = mybir.MatmulPerfMode.DoubleRow

            # ---- qk projection ----
            def qk_oct(t):
                # octile t: rows 128t..128t+128 of qk output; t<3 -> q64,
                # t>=3 -> k8; heads (2(t%3), 2(t%3)+1)
                qp = ps.tile([128, S], F32, tag="big", name="qp")
                if FP8_QKV:
                    for j in range(3):
                        for nh in range(2):
                            sl = 512 * nh
                            nc.tensor.matmul(
                                qp[:, sl:sl + 512],
                                _ap(wqkp_sb, 1536 * j + 128 * t,
                                    [[768, 2], [1, 128]]),
                                _ap(xTp_sb, 2048 * j + sl, [[1024, 2], [1, 512]]),
                                start=(j == 0), stop=(j == 2), perf_mode=DR)
                else:
                    for ci in range(6):
                        for nh in range(2):
                            sl = 512 * nh
                            nc.tensor.matmul(
                                qp[:, sl:sl + 512],
                                wqkp_sb[:, 768 * ci + 128 * t:
                                        768 * ci + 128 * (t + 1)],
                                xTp_sb[:, 1024 * ci + sl:1024 * ci + sl + 512],
                                start=(ci == 0), stop=(ci == 5))
                for sub in range(2):
                    head = (t % 3) * 2 + sub
                    dst = (qaug if t < 3 else kaug)[0:64, S * head:S * (head + 1)]
                    eng = nc.scalar.copy if t < 3 else nc.vector.tensor_copy
                    eng(dst, qp[64 * sub:64 * sub + 64, :])

            # ---- v projection ----
            def v_proj():
                for sb in range(8):
                    vp = ps.tile([128, 384], F32, tag="big", name="vp")
                    if FP8_QKV:
                        for j in range(3):
                            nc.tensor.matmul(
                                vp[:],
                                _ap(xTp_sb, 2048 * j + 128 * sb,
                                    [[1024, 2], [1, 128]]),
                                _ap(wvp_sb, 768 * j, [[384, 2], [1, 384]]),
                                start=(j == 0), stop=(j == 2), perf_mode=DR)
                    else:
                        for ci in range(6):
                            nc.tensor.matmul(
                                vp[:],
                                xTp_sb[:, 1024 * ci + 128 * sb:
                                       1024 * ci + 128 * (sb + 1)],
                                wvp_sb[:, 384 * ci:384 * (ci + 1)],
                                start=(ci == 0), stop=(ci == 5))
                    dst = _ap(vps, 1536 * (sb // 2) + 768 * (sb % 2),
                              [[128, HPC], [1, 64]])
                    src = _ap(vp, 0, [[64, HPC], [1, 64]])
                    nc.vector.tensor_copy(dst, src)
                for j in range(4):
                    nc.gpsimd.memset(
                        _ap(vps, 1536 * j + 64, [[768, 2], [128, HPC]]), 1.0)
                    nc.gpsimd.memset(
                        _ap(vps, 1536 * j + 65, [[768, 2], [128, HPC], [1, 63]]),
                        0.0)

            # ---- direct band extraction for a head pair ----
            def band(p):
                for ax, tbl in ((0, rh2_sb), (1, rw2_sb)):
                    for qt in range(4):
                        bt = ps.tile([32, 512], F32, tag="band", name="bt",
                                     bufs=2)
                        for s8 in range(8):
                            s = 8 * qt + s8
                            lhsT = tbl[:, 31 - s:63 - s]
                            if ax == 0:
                                rhs = _app(qaug, 0, 64, 2048 * p + 32 * s,
                                           [[1024, 2], [1, 32]])
                            else:
                                rhs = _app(qaug, 0, 64, 2048 * p + s,
                                           [[1024, 2], [32, 32]])
                            nc.tensor.matmul(
                                _ap(bt, 32 * s8, [[256, 2], [1, 32]]),
                                lhsT, rhs, start=True, stop=True)
                        for hh in range(2):
                            i = 2 * p + hh
                            eng = nc.vector.tensor_copy
                            if ax == 0:
                                eng(qaug[64:96, S * i + 256 * qt:
                                         S * i + 256 * (qt + 1)],
                                    bt[:, 256 * hh:256 * (hh + 1)])
                            else:
                                dst = _app(qaug, 96, 32, S * i + 8 * qt,
                                           [[32, 32], [1, 8]])
                                src = _ap(bt, 256 * hh, [[1, 32], [32, 8]])
                                eng(dst, src)

            # ---- attention for one head ----
            def attn(i):
                av = ps.tile([128, S], F32, tag="av", name="av", bufs=1)
                for j in range(4):
                    e = small.tile([128, 2048], BF16, tag="et", name="et",
                                   bufs=3)
                    for kb2 in range(2):
                        kb = 2 * j + kb2
                        sc = ps.tile([128, S], F32, tag="big", name="sc")
                        for nh in range(2):
                            sl = 512 * nh
                            nc.tensor.matmul(
                                sc[:, sl:sl + 512],
                                kaug[:, S * i + 128 * kb:S * i + 128 * (kb + 1)],
                                qaug[:, S * i + sl:S * i + sl + 512],
                                start=True, stop=True)
                        nc.scalar.activation(
                            e[:, 1024 * kb2:1024 * (kb2 + 1)], sc[:],
                            mybir.ActivationFunctionType.Exp, scale=EXP_SCALE)
                        for nh in range(2):
                            sl = 512 * nh
                            nc.tensor.matmul(
                                av[:, sl:sl + 512],
                                _ap(vps, 1536 * j + 768 * kb2 + 128 * i,
                                    [[1, 128]]),
                                _ap(e, 1024 * kb2 + sl, [[1, 512]]),
                                start=(kb == 0), stop=(kb == 7))
                avs = small.tile([65, S], F32, tag="avs", name="avs", bufs=2)
                nc.vector.tensor_copy(avs[0:65, :], av[0:65, :])
                rec = small.tile([1, S], F32, tag="rec", name="rec", bufs=2)
                nc.vector.reciprocal(rec[:], avs[64:65, :])
                rb = small.tile([64, S], F32, tag="rb", name="rb", bufs=2)
                nc.gpsimd.partition_broadcast(rb[:], rec[:])
                chunk, row = i // 2, (i % 2) * 64
                nc.vector.tensor_tensor(
                    ohT[row:row + 64, S * chunk:S * (chunk + 1)],
                    avs[0:64, :], rb[:], op=mybir.AluOpType.mult)

            # ---- schedule: stagger PE-only work between attention heads ----
            qk_oct(0); qk_oct(3)
            v_proj()
            band(0)
            qk_oct(1); qk_oct(4)
            attn(0)
            band(1)
            attn(1)
            qk_oct(2); qk_oct(5)
            attn(2)
            band(2)
            attn(3)
            attn(4)
            attn(5)

            # ---- output projection (bf16) ----
            for qb in range(8):
                pp = ps.tile([128, C], F32, tag="big", name="pp")
                for ci in range(3):
                    lhsT = ohT[:, S * ci + 128 * qb:S * ci + 128 * (qb + 1)]
                    nc.tensor.matmul(pp[:, 0:512], lhsT,
                                     wpr_sb[:, 768 * ci:768 * ci + 512],
                                     start=(ci == 0), stop=(ci == 2))
                    nc.tensor.matmul(pp[:, 512:768], lhsT,
                                     wpr_sb[:, 768 * ci + 512:768 * (ci + 1)],
                                     start=(ci == 0), stop=(ci == 2))
                pps = small.tile([128, C], F32, tag="pps", name="pps", bufs=2)
                (nc.scalar.copy if qb % 2 else nc.vector.tensor_copy)(
                    pps[:], pp[:])
                nc.sync.dma_start(out[128 * qb:128 * (qb + 1), :], pps[:])

    nc.compile()
    return nc


def shard_inputs(x, Wqkv, Wproj, rel_pos_h, rel_pos_w):
    """Build the 8 per-core input maps."""
    import ml_dtypes
    bf16 = ml_dtypes.bfloat16
    fp8 = ml_dtypes.float8_e4m3
    xdt = fp8 if FP8_QKV else bf16
    scale = HD ** (-0.5)
    x = np.asarray(x, dtype=np.float32)
    Wqkv = np.asarray(Wqkv, dtype=np.float32)
    Wproj = np.asarray(Wproj, dtype=np.float32)

    # flipped rel-pos tables, x64: rhTf[c, j] = 64 * rel_pos[62-j, c]
    rh2 = np.ascontiguousarray(
        (np.asarray(rel_pos_h, np.float32).T[:, ::-1] * 64.0)).astype(bf16)
    rw2 = np.ascontiguousarray(
        (np.asarray(rel_pos_w, np.float32).T[:, ::-1] * 64.0)).astype(bf16)

    # one-hot selector rows for kaug rows 64-127
    ohk = np.zeros((64, S), np.float32)
    kh = np.arange(S) // W
    kw = np.arange(S) % W
    ohk[kh, np.arange(S)] = 1.0
    ohk[32 + kw, np.arange(S)] = 1.0
    ohk = ohk.astype(bf16)

    def lay(a):
        # (768, M) -> SBUF image (128, 6M)
        M = a.shape[1]
        if FP8_QKV:
            # pair-interleaved (128, 3, 2, M) for DoubleRow
            r = a.reshape(3, 2, 128, M).transpose(2, 0, 1, 3)
        else:
            r = a.reshape(6, 128, M).transpose(1, 0, 2)
        return np.ascontiguousarray(r.reshape(128, 6 * M)).astype(xdt)

    in_maps = []
    for core in range(NCORES):
        b = core // 2
        h0 = (core % 2) * HPC
        xb = x[b].reshape(S, C)
        xT = np.ascontiguousarray(xb.T)
        wq = Wqkv[:, h0 * HD:(h0 + HPC) * HD] * 64.0
        wk = Wqkv[:, C + h0 * HD:C + (h0 + HPC) * HD] * (64.0 * scale)
        wqk = np.concatenate([wq, wk], axis=1)
        wv = Wqkv[:, 2 * C + h0 * HD:2 * C + (h0 + HPC) * HD] * 64.0
        wp = np.ascontiguousarray(
            Wproj[h0 * HD:(h0 + HPC) * HD, :] / 64.0).astype(bf16)
        in_maps.append({"xTp": lay(xT), "wqkp": lay(wqk), "wvp": lay(wv),
                        "wpr": wp, "rh2": rh2, "rw2": rw2, "ohk": ohk})
    return in_maps


_NC_CACHE = {}


def kernel(x, Wqkv, Wproj, bproj, rel_pos_h, rel_pos_w):
    if "nc" not in _NC_CACHE:
        _NC_CACHE["nc"] = build_program()
    nc = _NC_CACHE["nc"]
    in_maps = shard_inputs(x, Wqkv, Wproj, rel_pos_h, rel_pos_w)
    res = run_bass_kernel_spmd(nc, in_maps, list(range(NCORES)))
    bproj = np.asarray(bproj, dtype=np.float32)
    out = np.empty((B, H, W, C), dtype=np.float32)
    for b in range(B):
        acc = res.results[2 * b]["out"] + res.results[2 * b + 1]["out"] + bproj
        out[b] = acc.reshape(H, W, C)
    return out
